# revision 6
# baseline (speedup 1.0000x reference)
"""Multi-head self-attention Trainium2 kernel (8 NeuronCores).

Sharding: 8 cores = 4 batches x 2 head-halves (Megatron-style tensor
parallel over heads within a batch). Each core computes, for its batch b
and its 6 heads:
    Q^T, K^T (head-dim-on-partitions layout), V (natural layout,
    ones-column augmented), S^T = K.Q^T per (head, q-block), P = exp(S/8)
    (softmax without max subtraction -- scores are O(5), safe in fp32),
    ctx^T = (V^T P^T) with the softmax denominator arriving as the
    ones-row, normalization fused into the PSUM drain, and finally the
    out-projection Y_partial = ctx @ Wo[:, cols]^T.
The host sums the two per-batch partials and adds the output bias.

All matmul operands are bf16 (fp32 PSUM accumulation); exp and
normalization run in fp32.
"""

import os
import numpy as np

B, S, E, H, D = 4, 2048, 768, 12, 64
NCORES = 8

_FULL_CFG = dict(S=2048, E=768, EL=384)

_cache = {}


def _emit(nc, tc, ctx, io, cfg):
    import concourse.bass as bass
    import concourse.mybir as mybir

    fp32 = mybir.dt.float32
    bf16 = mybir.dt.bfloat16
    Exp = mybir.ActivationFunctionType.Exp

    Scfg, Ecfg, EL = cfg["S"], cfg["E"], cfg["EL"]
    ROT = cfg.get("rot", 3)    # exp group size in PSUM banks
    NKT = Ecfg // 128          # contraction tiles over embed dim
    NS = Scfg // 128           # sequence tiles (also key tiles)
    NQB = Scfg // 512          # query blocks
    HP = EL // 128             # head pairs (2 heads each)
    NCOMBO = 2 * NS            # (head, k-tile) combos per unit
    NROT = (NCOMBO + ROT - 1) // ROT   # exp rotations
    NYC = (Ecfg + 383) // 384  # out-proj column chunks
    YCW = Ecfg // NYC          # chunk width
    scale = 1.0 / np.sqrt(D)

    XT, WQT, WKT, WVT, WOT, BQ, BK, BV, Y = (
        io["XT"], io["WQT"], io["WKT"], io["WVT"], io["WOT"],
        io["BQ"], io["BK"], io["BV"], io["Y"],
    )

    consts = ctx.enter_context(tc.tile_pool(name="consts", bufs=1))
    wpool = ctx.enter_context(tc.tile_pool(name="wpool", bufs=1))
    xpool = ctx.enter_context(tc.tile_pool(name="xpool", bufs=1))
    qkpool = ctx.enter_context(tc.tile_pool(name="qkpool", bufs=2))
    vpool = ctx.enter_context(tc.tile_pool(name="vpool", bufs=2))
    spool = ctx.enter_context(tc.tile_pool(name="spool", bufs=16))
    cpool = ctx.enter_context(tc.tile_pool(name="cpool", bufs=1))
    rpool = ctx.enter_context(tc.tile_pool(name="rpool", bufs=2))
    ypool = ctx.enter_context(tc.tile_pool(name="ypool", bufs=3))
    psum_g = ctx.enter_context(tc.tile_pool(name="psum_g", bufs=2, space="PSUM"))
    psum_c = ctx.enter_context(tc.tile_pool(name="psum_c", bufs=1, space="PSUM"))
    psum_m = ctx.enter_context(tc.tile_pool(name="psum_m", bufs=1, space="PSUM"))

    # ---- constants & weights to SBUF ----
    bq_sb = consts.tile([128, HP], fp32, name="bq_sb")
    nc.sync.dma_start(out=bq_sb, in_=BQ)
    bk_sb = consts.tile([128, HP], fp32, name="bk_sb")
    nc.sync.dma_start(out=bk_sb, in_=BK)
    bv_sb = consts.tile([128, EL], fp32, name="bv_sb")
    nc.sync.dma_start(out=bv_sb, in_=BV)

    wq_sb, wk_sb, wv_sb = [], [], []
    for kk in range(NKT):
        for lst, src, nm in ((wq_sb, WQT, "wq"), (wk_sb, WKT, "wk"), (wv_sb, WVT, "wv")):
            t = wpool.tile([128, EL], bf16, name=f"{nm}{kk}_sb", tag=f"{nm}{kk}")
            nc.sync.dma_start(out=t, in_=src[kk * 128:(kk + 1) * 128, :])
            lst.append(t)
    wo_sb = []
    for hp in range(HP):
        t = wpool.tile([128, Ecfg], bf16, name=f"wo{hp}_sb", tag=f"wo{hp}")
        nc.sync.dma_start(out=t, in_=WOT[hp * 128:(hp + 1) * 128, :])
        wo_sb.append(t)

    xt_sb = []
    for kk in range(NKT):
        t = xpool.tile([128, Scfg], bf16, name=f"xt{kk}_sb", tag=f"xt{kk}")
        nc.sync.dma_start(out=t, in_=XT[kk * 128:(kk + 1) * 128, :])
        xt_sb.append(t)

    ctxT = []
    for hp in range(HP):
        t = cpool.tile([128, Scfg], bf16, name=f"ctxT{hp}", tag=f"ctx{hp}")
        ctxT.append(t)

    # ---- per head-pair: projections then attention units ----
    for hp in range(HP):
        # Q^T / K^T for this pair: (128 e_local, S) bf16
        qt = qkpool.tile([128, Scfg], bf16, name=f"qt{hp}", tag="qt")
        kt = qkpool.tile([128, Scfg], bf16, name=f"kt{hp}", tag="kt")
        pp = 0
        for dst, wsb, bsb in ((qt, wq_sb, bq_sb), (kt, wk_sb, bk_sb)):
            for nb in range(NQB):
                pool = psum_m if (hp > 0 or pp % 2 == 0) else psum_c
                ps = pool.tile([128, 512], fp32, name=f"pj{hp}_{pp}",
                               tag="m" if pool is psum_m else "c")
                pp += 1
                for kk in range(NKT):
                    nc.tensor.matmul(
                        ps, lhsT=wsb[kk][:, hp * 128:(hp + 1) * 128],
                        rhs=xt_sb[kk][:, nb * 512:(nb + 1) * 512],
                        start=(kk == 0), stop=(kk == NKT - 1))
                nc.vector.tensor_scalar_add(
                    dst[:, nb * 512:(nb + 1) * 512], ps, bsb[:, hp:hp + 1])

        # V tiles, natural layout, per head [d0..d63 | 1] (65 cols/head)
        v_sb = []
        for si in range(NS):
            vt = vpool.tile([128, 130], bf16, name=f"v{hp}_{si}", tag=f"v{si}")
            nc.vector.memset(vt, 1.0)
            pool = psum_m if (hp > 0 or pp % 2 == 0) else psum_c
            ps = pool.tile([128, 128], fp32, name=f"pv{hp}_{si}",
                           tag="m" if pool is psum_m else "c")
            pp += 1
            for kk in range(NKT):
                nc.tensor.matmul(
                    ps, lhsT=xt_sb[kk][:, si * 128:(si + 1) * 128],
                    rhs=wv_sb[kk][:, hp * 128:(hp + 1) * 128],
                    start=(kk == 0), stop=(kk == NKT - 1))
            nc.vector.tensor_add(
                vt.rearrange("p (h w) -> p h w", w=65)[:, :, 0:64],
                ps.rearrange("p (h w) -> p h w", w=64),
                bv_sb[:, hp * 128:(hp + 1) * 128].rearrange(
                    "p (h w) -> p h w", w=64))
            v_sb.append(vt)

        # attention units: one per query block
        for qb in range(0 if not cfg.get("skip_attn") else NQB, NQB):
            qsl = slice(qb * 512, qb * 512 + 512)
            slabs = []
            cps = None

            def emit_ctx(ci):
                nonlocal cps
                hh, k = divmod(ci, NS)
                if k == 0:
                    cps = psum_c.tile([128, 512], fp32,
                                      name=f"cps{hp}_{qb}_{hh}", tag="c")
                nc.tensor.matmul(
                    cps[0:65, :],
                    lhsT=v_sb[k][:, hh * 65:(hh + 1) * 65],
                    rhs=slabs[ci // ROT][:, (ci % ROT) * 512:(ci % ROT) * 512 + 512],
                    start=(k == 0), stop=(k == NS - 1))
                if k == NS - 1:
                    r_sb = rpool.tile([1, 512], fp32,
                                      name=f"r{hp}_{qb}_{hh}", tag="r")
                    nc.vector.reciprocal(r_sb, cps[64:65, :])
                    rb = rpool.tile([128, 512], fp32,
                                    name=f"rb{hp}_{qb}_{hh}", tag="rb")
                    nc.gpsimd.partition_broadcast(rb, r_sb)
                    nc.vector.tensor_mul(
                        ctxT[hp][64 * hh:64 * hh + 64, qsl],
                        cps[0:64, :], rb[0:64, :])

            for r in range(NROT):
                cis = range(ROT * r, min(ROT * r + ROT, NCOMBO))
                n = len(cis)
                g = psum_g.tile([128, 512 * ROT], fp32, name=f"g{hp}_{qb}_{r}", tag="g")
                for j, ci in enumerate(cis):
                    hh, k = divmod(ci, NS)
                    nc.tensor.matmul(
                        g[:, j * 512:(j + 1) * 512],
                        lhsT=kt[hh * 64:(hh + 1) * 64, k * 128:(k + 1) * 128],
                        rhs=qt[hh * 64:(hh + 1) * 64, qsl],
                        start=True, stop=True)
                slab = spool.tile([128, 512 * ROT], bf16, name=f"s{hp}_{qb}_{r}", tag="slab")
                nc.scalar.activation(slab[:, :n * 512], g[:, :n * 512], Exp,
                                     scale=float(scale))
                slabs.append(slab)
                if r >= 2:
                    for ci in range(ROT * (r - 2), ROT * (r - 1)):
                        emit_ctx(ci)
            for ci in range(max(0, ROT * (NROT - 2)), NCOMBO):
                emit_ctx(ci)

    # ---- out-projection: Y = ctx @ Wo_loc^T ----
    for si in range(0 if not cfg.get("skip_y") else NS, NS):
        y_sb = ypool.tile([128, Ecfg], fp32, name=f"y{si}", tag="y")
        for nh in range(NYC):
            yps = psum_g.tile([128, YCW], fp32, name=f"yp{si}_{nh}", tag="g")
            for hp in range(HP):
                nc.tensor.matmul(
                    yps, lhsT=ctxT[hp][:, si * 128:(si + 1) * 128],
                    rhs=wo_sb[hp][:, nh * YCW:(nh + 1) * YCW],
                    start=(hp == 0), stop=(hp == HP - 1))
            nc.vector.tensor_copy(y_sb[:, nh * YCW:(nh + 1) * YCW], yps)
        nc.sync.dma_start(out=Y[si * 128:(si + 1) * 128, :], in_=y_sb)


def _build(cfg):
    import contextlib
    import concourse.mybir as mybir
    import concourse.tile as tile
    from concourse import bacc

    Scfg, Ecfg, EL = cfg["S"], cfg["E"], cfg["EL"]
    HP = EL // 128
    fp32, bf16 = mybir.dt.float32, mybir.dt.bfloat16

    nc = bacc.Bacc("TRN2", target_bir_lowering=False, debug=False,
                   num_devices=NCORES)
    io = {
        "XT": nc.dram_tensor("XT", [Ecfg, Scfg], bf16, kind="ExternalInput").ap(),
        "WQT": nc.dram_tensor("WQT", [Ecfg, EL], bf16, kind="ExternalInput").ap(),
        "WKT": nc.dram_tensor("WKT", [Ecfg, EL], bf16, kind="ExternalInput").ap(),
        "WVT": nc.dram_tensor("WVT", [Ecfg, EL], bf16, kind="ExternalInput").ap(),
        "WOT": nc.dram_tensor("WOT", [EL, Ecfg], bf16, kind="ExternalInput").ap(),
        "BQ": nc.dram_tensor("BQ", [128, HP], fp32, kind="ExternalInput").ap(),
        "BK": nc.dram_tensor("BK", [128, HP], fp32, kind="ExternalInput").ap(),
        "BV": nc.dram_tensor("BV", [128, EL], fp32, kind="ExternalInput").ap(),
        "Y": nc.dram_tensor("Y", [Scfg, Ecfg], fp32, kind="ExternalOutput").ap(),
    }
    with tile.TileContext(nc) as tc:
        with contextlib.ExitStack() as ctx:
            _emit(nc, tc, ctx, io, cfg)
    nc.compile()
    return nc


def _get_program(cfg_key="full"):
    if cfg_key not in _cache:
        _cache[cfg_key] = _build(_FULL_CFG)
    return _cache[cfg_key]


def _core_inputs(c, X, Wq, bq, Wk, bk, Wv, bv):
    import ml_dtypes
    bf16 = ml_dtypes.bfloat16
    b, half = divmod(c, 2)
    e0 = 384 * half
    ecols = slice(e0, e0 + 384)
    return {
        "XT": np.ascontiguousarray(X[b].T).astype(bf16),
        "WQT": np.ascontiguousarray(Wq[ecols, :].T).astype(bf16),
        "WKT": np.ascontiguousarray(Wk[ecols, :].T).astype(bf16),
        "WVT": np.ascontiguousarray(Wv[ecols, :].T).astype(bf16),
        "BQ": np.ascontiguousarray(bq[ecols].reshape(3, 128).T).astype(np.float32),
        "BK": np.ascontiguousarray(bk[ecols].reshape(3, 128).T).astype(np.float32),
        "BV": np.ascontiguousarray(
            np.broadcast_to(bv[ecols], (128, 384))).astype(np.float32),
    }


def kernel(X, Wq, bq, Wk, bk, Wv, bv, Wo, bo):
    import ml_dtypes
    from concourse.bass_utils import run_bass_kernel_spmd

    bf16 = ml_dtypes.bfloat16
    X, Wq, bq, Wk, bk, Wv, bv, Wo, bo = [
        np.asarray(a, dtype=np.float32)
        for a in (X, Wq, bq, Wk, bk, Wv, bv, Wo, bo)
    ]
    nc = _get_program()
    in_maps = []
    for c in range(NCORES):
        m = _core_inputs(c, X, Wq, bq, Wk, bk, Wv, bv)
        half = c % 2
        ecols = slice(384 * half, 384 * half + 384)
        m["WOT"] = np.ascontiguousarray(Wo[:, ecols].T).astype(bf16)
        in_maps.append(m)
    res = run_bass_kernel_spmd(nc, in_maps, list(range(NCORES)))
    out = np.empty((B, S, E), np.float32)
    for b in range(B):
        out[b] = (res.results[2 * b]["Y"] + res.results[2 * b + 1]["Y"]
                  + bo[None, :])
    return out


# revision 21
# speedup vs baseline: 1.0248x; 1.0248x over previous
"""Multi-head self-attention Trainium2 kernel (8 NeuronCores).

Sharding: 8 cores = 4 batches x 2 head-halves (Megatron-style tensor
parallel over heads within a batch). Each core computes, for its batch b
and its 6 heads:
    Q^T, K^T (head-dim-on-partitions layout), V (natural layout,
    ones-column augmented), S^T = K.Q^T per (head, q-block), P = exp(S/8)
    (softmax without max subtraction -- scores are O(5), safe in fp32),
    ctx^T = (V^T P^T) with the softmax denominator arriving as the
    ones-row, normalization fused into the PSUM drain, and finally the
    out-projection Y_partial = ctx @ Wo[:, cols]^T.
The host sums the two per-batch partials and adds the output bias.

All matmul operands are bf16 (fp32 PSUM accumulation); exp and
normalization run in fp32.
"""

import numpy as np

B, S, E, H, D = 4, 2048, 768, 12, 64
NCORES = 8

_FULL_CFG = dict(S=2048, E=768, EL=384)

_cache = {}


def _emit(nc, tc, ctx, io, cfg):
    import concourse.mybir as mybir

    fp32 = mybir.dt.float32
    bf16 = mybir.dt.bfloat16
    Exp = mybir.ActivationFunctionType.Exp

    Scfg, Ecfg, EL = cfg["S"], cfg["E"], cfg["EL"]
    ROT = cfg.get("rot", 3)    # exp group size in PSUM banks
    NKT = Ecfg // 128          # contraction tiles over embed dim
    NS = Scfg // 128           # sequence tiles (also key tiles)
    NQB = Scfg // 512          # query blocks
    HP = EL // 128             # head pairs (2 heads each)
    NCOMBO = 2 * NS            # (head, k-tile) combos per unit
    NROT = (NCOMBO + ROT - 1) // ROT   # exp rotations
    NYC = (Ecfg + 383) // 384  # out-proj column chunks
    YCW = Ecfg // NYC          # chunk width
    scale = 1.0 / np.sqrt(D)

    XT, WQT, WKT, WVT, WOT, BQ, BK, BV, Y = (
        io["XT"], io["WQT"], io["WKT"], io["WVT"], io["WOT"],
        io["BQ"], io["BK"], io["BV"], io["Y"],
    )

    consts = ctx.enter_context(tc.tile_pool(name="consts", bufs=1))
    wpool = ctx.enter_context(tc.tile_pool(name="wpool", bufs=1))
    xpool = ctx.enter_context(tc.tile_pool(name="xpool", bufs=1))
    qkpool = ctx.enter_context(tc.tile_pool(name="qkpool", bufs=2))
    vpool = ctx.enter_context(tc.tile_pool(name="vpool", bufs=2))
    spool = ctx.enter_context(tc.tile_pool(name="spool", bufs=22))
    cpool = ctx.enter_context(tc.tile_pool(name="cpool", bufs=1))
    rpool = ctx.enter_context(tc.tile_pool(name="rpool", bufs=2))
    ypool = ctx.enter_context(tc.tile_pool(name="ypool", bufs=3))
    psum_g = ctx.enter_context(tc.tile_pool(name="psum_g", bufs=2, space="PSUM"))
    psum_c = ctx.enter_context(tc.tile_pool(name="psum_c", bufs=1, space="PSUM"))
    psum_m = ctx.enter_context(tc.tile_pool(name="psum_m", bufs=1, space="PSUM"))

    # ---- constants & weights to SBUF (critical-path tensors first: the
    # first projection chain needs all XT tiles plus Wq/Wk) ----
    xt_sb = []
    for kk in range(NKT):
        t = xpool.tile([128, Scfg], bf16, name=f"xt{kk}_sb", tag=f"xt{kk}")
        nc.sync.dma_start(out=t, in_=XT[kk * 128:(kk + 1) * 128, :])
        xt_sb.append(t)

    wq_sb, wk_sb, wv_sb = [], [], []
    for kk in range(NKT):
        for lst, src, nm in ((wq_sb, WQT, "wq"), (wk_sb, WKT, "wk"), (wv_sb, WVT, "wv")):
            t = wpool.tile([128, EL], bf16, name=f"{nm}{kk}_sb", tag=f"{nm}{kk}")
            nc.sync.dma_start(out=t, in_=src[kk * 128:(kk + 1) * 128, :])
            lst.append(t)

    bq_sb = consts.tile([128, HP], fp32, name="bq_sb")
    nc.sync.dma_start(out=bq_sb, in_=BQ)
    bk_sb = consts.tile([128, HP], fp32, name="bk_sb")
    nc.sync.dma_start(out=bk_sb, in_=BK)
    bv_sb = consts.tile([128, EL], fp32, name="bv_sb")
    nc.sync.dma_start(out=bv_sb, in_=BV)

    wo_sb = []
    for hp in range(HP):
        t = wpool.tile([128, Ecfg], bf16, name=f"wo{hp}_sb", tag=f"wo{hp}")
        nc.sync.dma_start(out=t, in_=WOT[hp * 128:(hp + 1) * 128, :])
        wo_sb.append(t)

    ctxT = []
    for hp in range(HP):
        t = cpool.tile([128, Scfg], bf16, name=f"ctxT{hp}", tag=f"ctx{hp}")
        ctxT.append(t)

    # ctx for unit u runs while unit u+1's score rotations stream, keeping
    # ACT saturated. Combos are k-major (ci = 2k+hh) so the two heads' K=64
    # score matmuls land in opposite row-halves of the PE array and run
    # concurrently.
    def ctx_combo(u, idx):
        hh, k = divmod(idx, NS)
        ci = 2 * k + hh
        if k == 0:
            u["cps"] = psum_c.tile([128, 512], fp32,
                                   name=f"cps{u['hp']}_{u['qb']}_{hh}", tag="c")
        cps = u["cps"]
        vt = u["v"][k]
        rhs = u["slabs"][ci // ROT][:, (ci % ROT) * 512:(ci % ROT) * 512 + 512]
        nc.tensor.matmul(cps[0:65, :], lhsT=vt[:, hh * 65:hh * 65 + 65],
                         rhs=rhs, start=(k == 0), stop=(k == NS - 1))
        if k == NS - 1:
            hp_u, qb_u = u["hp"], u["qb"]
            qsl = slice(qb_u * 512, qb_u * 512 + 512)
            r_sb = rpool.tile([1, 512], fp32, name=f"r{hp_u}_{qb_u}_{hh}", tag="r")
            nc.vector.reciprocal(r_sb, cps[64:65, :])
            rb = rpool.tile([128, 512], fp32, name=f"rb{hp_u}_{qb_u}_{hh}", tag="rb")
            nc.gpsimd.partition_broadcast(rb, r_sb)
            nc.vector.tensor_mul(
                ctxT[hp_u][64 * hh:64 * hh + 64, qsl],
                cps[0:64, :], rb[0:64, :])

    prev_unit = None
    import collections
    pending = collections.deque()

    def drip(n=1):
        for _ in range(min(n, len(pending))):
            pending.popleft()()

    # ---- projection closures (one PSUM slice each) so they can be dripped
    # into the attention rotation stream of the previous head pair ----
    def build_proj(hp):
        qt = qkpool.tile([128, Scfg], bf16, name=f"qt{hp}", tag="qt")
        kt = qkpool.tile([128, Scfg], bf16, name=f"kt{hp}", tag="kt")
        v_sb = [vpool.tile([128, 130], bf16, name=f"v{hp}_{si}", tag=f"v{si}")
                for si in range(NS)]
        closures = []
        for lbl, dst, wsb, bsb in (("q", qt, wq_sb, bq_sb), ("k", kt, wk_sb, bk_sb)):
            for nb in range(NQB):
                def cl(pool, lbl=lbl, dst=dst, wsb=wsb, bsb=bsb, nb=nb, hp=hp):
                    ps = pool.tile([128, 512], fp32,
                                   name=f"pj{hp}{lbl}_{nb}",
                                   tag="m" if pool is psum_m else "c")
                    for kk in range(NKT):
                        nc.tensor.matmul(
                            ps, lhsT=wsb[kk][:, hp * 128:(hp + 1) * 128],
                            rhs=xt_sb[kk][:, nb * 512:(nb + 1) * 512],
                            start=(kk == 0), stop=(kk == NKT - 1))
                    nc.vector.tensor_scalar_add(
                        dst[:, nb * 512:(nb + 1) * 512], ps, bsb[:, hp:hp + 1])
                closures.append(cl)
        for si in range(NS):
            def cl(pool, si=si, hp=hp, vt=v_sb[si]):
                nc.vector.memset(vt, 1.0)
                ps = pool.tile([128, 128], fp32, name=f"pv{hp}_{si}",
                               tag="m" if pool is psum_m else "c")
                for kk in range(NKT):
                    nc.tensor.matmul(
                        ps, lhsT=xt_sb[kk][:, si * 128:(si + 1) * 128],
                        rhs=wv_sb[kk][:, hp * 128:(hp + 1) * 128],
                        start=(kk == 0), stop=(kk == NKT - 1))
                nc.vector.tensor_add(
                    vt.rearrange("p (h w) -> p h w", w=65)[:, :, 0:64],
                    ps.rearrange("p (h w) -> p h w", w=64),
                    bv_sb[:, hp * 128:(hp + 1) * 128].rearrange(
                        "p (h w) -> p h w", w=64))
            closures.append(cl)
        return qt, kt, v_sb, closures

    # ---- out-projection closures: Y = ctx @ Wo_loc^T, dripped into the
    # last head pair's attention stream once the needed ctxT columns are
    # fully drained ----
    def build_y(si):
        def cl(pool, si=si):
            y_sb = ypool.tile([128, Ecfg], fp32, name=f"y{si}", tag="y")
            for nh in range(NYC):
                yps = pool.tile([128, YCW], fp32, name=f"yp{si}_{nh}",
                                tag="m" if pool is psum_m else "c")
                for hp in range(HP):
                    nc.tensor.matmul(
                        yps, lhsT=ctxT[hp][:, si * 128:(si + 1) * 128],
                        rhs=wo_sb[hp][:, nh * YCW:(nh + 1) * YCW],
                        start=(hp == 0), stop=(hp == HP - 1))
                nc.vector.tensor_copy(y_sb[:, nh * YCW:(nh + 1) * YCW], yps)
            nc.sync.dma_start(out=Y[si * 128:(si + 1) * 128, :], in_=y_sb)
        return cl

    y_by_qb = {qb: [build_y(si) for si in range(qb * (NS // NQB),
                                                (qb + 1) * (NS // NQB))]
               for qb in range(NQB)}

    y_sched = set()
    cur = build_proj(0)
    for hp in range(HP):
        qt, kt, v_sb, closures = cur
        if hp == 0:
            for i, cl in enumerate(closures):
                cl(psum_m if i % 2 == 0 else psum_c)
        cur = build_proj(hp + 1) if hp + 1 < HP else None
        if cur is not None:
            pending.extend((lambda cl=cl: cl(psum_m)) for cl in cur[3])

        for qb in range(NQB):
            qsl = slice(qb * 512, qb * 512 + 512)
            if hp == HP - 1 and qb >= 2:
                y_sched.add(qb - 2)
                pending.extend(
                    (lambda cl=cl: cl(psum_m)) for cl in y_by_qb[qb - 2])
            unit = dict(hp=hp, qb=qb, v=v_sb, slabs=[], cps=None, cur=0)
            for r in range(NROT):
                cis = range(ROT * r, min(ROT * r + ROT, NCOMBO))
                n = len(cis)
                g = psum_g.tile([128, 512 * ROT], fp32,
                                name=f"g{hp}_{qb}_{r}", tag="g")
                for j, ci in enumerate(cis):
                    hh, k = ci % 2, ci // 2
                    nc.tensor.matmul(
                        g[:, j * 512:(j + 1) * 512],
                        lhsT=kt[hh * 64:(hh + 1) * 64, k * 128:(k + 1) * 128],
                        rhs=qt[hh * 64:(hh + 1) * 64, qsl],
                        start=True, stop=True)
                slab = spool.tile([128, 512 * ROT], bf16,
                                  name=f"s{hp}_{qb}_{r}", tag="slab")
                nc.scalar.activation(slab[:, :n * 512], g[:, :n * 512], Exp,
                                     scale=float(scale))
                unit["slabs"].append(slab)
                if prev_unit is not None:
                    target = min(NCOMBO, ROT * (r + 1))
                    while prev_unit["cur"] < target:
                        ctx_combo(prev_unit, prev_unit["cur"])
                        prev_unit["cur"] += 1
                drip(1)
            if prev_unit is not None:
                while prev_unit["cur"] < NCOMBO:
                    ctx_combo(prev_unit, prev_unit["cur"])
                    prev_unit["cur"] += 1
            prev_unit = unit

    # drain the last unit's ctx, dripping in remaining out-projections
    if NQB >= 2:
        y_sched.add(NQB - 2)
        pending.extend((lambda cl=cl: cl(psum_m)) for cl in y_by_qb[NQB - 2])
    if prev_unit is not None:
        while prev_unit["cur"] < NCOMBO:
            ctx_combo(prev_unit, prev_unit["cur"])
            prev_unit["cur"] += 1
            if prev_unit["cur"] % 3 == 0:
                drip(1)
    tail = [cl for qb in range(NQB) if qb not in y_sched
            for cl in y_by_qb[qb]]
    for i, cl in enumerate(tail):
        cl(psum_m if i % 2 == 0 else psum_c)
    drip(len(pending))


def _build(cfg):
    import contextlib
    import concourse.mybir as mybir
    import concourse.tile as tile
    from concourse import bacc

    Scfg, Ecfg, EL = cfg["S"], cfg["E"], cfg["EL"]
    HP = EL // 128
    fp32, bf16 = mybir.dt.float32, mybir.dt.bfloat16

    nc = bacc.Bacc("TRN2", target_bir_lowering=False, debug=False,
                   num_devices=NCORES)
    io = {
        "XT": nc.dram_tensor("XT", [Ecfg, Scfg], bf16, kind="ExternalInput").ap(),
        "WQT": nc.dram_tensor("WQT", [Ecfg, EL], bf16, kind="ExternalInput").ap(),
        "WKT": nc.dram_tensor("WKT", [Ecfg, EL], bf16, kind="ExternalInput").ap(),
        "WVT": nc.dram_tensor("WVT", [Ecfg, EL], bf16, kind="ExternalInput").ap(),
        "WOT": nc.dram_tensor("WOT", [EL, Ecfg], bf16, kind="ExternalInput").ap(),
        "BQ": nc.dram_tensor("BQ", [128, HP], fp32, kind="ExternalInput").ap(),
        "BK": nc.dram_tensor("BK", [128, HP], fp32, kind="ExternalInput").ap(),
        "BV": nc.dram_tensor("BV", [128, EL], fp32, kind="ExternalInput").ap(),
        "Y": nc.dram_tensor("Y", [Scfg, Ecfg], fp32, kind="ExternalOutput").ap(),
    }
    with tile.TileContext(nc) as tc:
        with contextlib.ExitStack() as ctx:
            _emit(nc, tc, ctx, io, cfg)
    nc.compile()
    return nc


def _get_program(cfg_key="full"):
    if cfg_key not in _cache:
        _cache[cfg_key] = _build(_FULL_CFG)
    return _cache[cfg_key]


def _core_inputs(c, X, Wq, bq, Wk, bk, Wv, bv):
    import ml_dtypes
    bf16 = ml_dtypes.bfloat16
    b, half = divmod(c, 2)
    e0 = 384 * half
    ecols = slice(e0, e0 + 384)
    return {
        "XT": np.ascontiguousarray(X[b].T).astype(bf16),
        "WQT": np.ascontiguousarray(Wq[ecols, :].T).astype(bf16),
        "WKT": np.ascontiguousarray(Wk[ecols, :].T).astype(bf16),
        "WVT": np.ascontiguousarray(Wv[ecols, :].T).astype(bf16),
        "BQ": np.ascontiguousarray(bq[ecols].reshape(3, 128).T).astype(np.float32),
        "BK": np.ascontiguousarray(bk[ecols].reshape(3, 128).T).astype(np.float32),
        "BV": np.ascontiguousarray(
            np.broadcast_to(bv[ecols], (128, 384))).astype(np.float32),
    }


def kernel(X, Wq, bq, Wk, bk, Wv, bv, Wo, bo):
    import ml_dtypes
    from concourse.bass_utils import run_bass_kernel_spmd

    bf16 = ml_dtypes.bfloat16
    X, Wq, bq, Wk, bk, Wv, bv, Wo, bo = [
        np.asarray(a, dtype=np.float32)
        for a in (X, Wq, bq, Wk, bk, Wv, bv, Wo, bo)
    ]
    nc = _get_program()
    in_maps = []
    for c in range(NCORES):
        m = _core_inputs(c, X, Wq, bq, Wk, bk, Wv, bv)
        half = c % 2
        ecols = slice(384 * half, 384 * half + 384)
        m["WOT"] = np.ascontiguousarray(Wo[:, ecols].T).astype(bf16)
        in_maps.append(m)
    res = run_bass_kernel_spmd(nc, in_maps, list(range(NCORES)))
    out = np.empty((B, S, E), np.float32)
    for b in range(B):
        out[b] = (res.results[2 * b]["Y"] + res.results[2 * b + 1]["Y"]
                  + bo[None, :])
    return out
